# revision 27
# baseline (speedup 1.0000x reference)
"""Bass/Tile Trainium2 kernel for additive (Bahdanau/'cat') attention.

Problem (per batch b):
  A[i,d]      = sum_a context[i,a] * attn_w[a,d] + attn_b[d]
  O[o,d]      = sum_e output[o,e]  * dec_w[e,d]  + dec_b[d]
  scores[o,i] = sum_d query_w[d] * tanh(A[i,d] + O[o,d])   (+query_b: softmax-invariant)
  attn        = softmax_i(scores)
  mix[o,a]    = sum_i attn[o,i] * context[i,a]
  out[o,d]    = tanh([mix | output] @ out_w + out_b)

Sharding: pure data-parallel over batch, B=8 -> one batch per NeuronCore,
weights broadcast, no collectives.

The 16.8M-element tanh stream on the ACT engine (~113us busy) is the hard
floor on this silicon (DVE reciprocal is ~6 cyc/elem, tensor_tensor divide
is invalid ISA, Pool is ~7x slower than DVE -> no second engine can take a
share of the nonlinearity).  Everything else is arranged AROUND that stream:
  * all matmul operands are cast to bf16 (and transposed) on the HOST -> the
    critical DMA volume halves and there are no on-device weight casts at
    all; the first tanh fires ~15us in instead of ~30us.
  * A^T/O^T bias adds run on DVE (tensor_scalar from PSUM), ACT runs tanh
    back-to-back; first/last units split for a fast start / short tail.
  * quarter epilogues: softmax+mix for rows 48..63 fire between group-3
    units, rows 32..47 after group 3 -> short tail (row-ops for rows 48..63
    run on the 32-aligned block since engine writes/PE transposes must
    start at partition 0/32/64/96).
  * keepalive ident-matmuls hold the PE p-state through group 3 and the
    final projection.
"""

import numpy as np
import ml_dtypes

import concourse.bass as bass
import concourse.tile as tile
import concourse.bass_utils as bass_utils
from concourse import bacc, mybir
from concourse.masks import make_identity

B, OUT_LEN, IN_LEN, DEC, ATTN = 8, 64, 512, 512, 512
P = 128
F32 = mybir.dt.float32
BF16 = mybir.dt.bfloat16
AF = mybir.ActivationFunctionType
BF16_NP = ml_dtypes.bfloat16

G = 16                    # o's per matmul group
NG = OUT_LEN // G         # 4 groups
DC = DEC // P             # 4 d-chunks
AC = ATTN // P            # 4 a-chunks
IC = IN_LEN // P          # 4 i-chunks
EC = DEC // P             # 4 e-chunks (decoder feature)
CC = (ATTN + DEC) // P    # 8 combined chunks
H = OUT_LEN // 2          # row half

N_CORES = 8


def _epilogue_softmax_mix(nc, r0, nr, ident_bf, scores_sb, exp_sb, sums, recip,
                          attn_sb, attn_bf, attnT_bf, ctx_bf, mixT_bf, psum,
                          attn_d):
    """softmax + attn^T + mix for rows r0..r0+nr (all-bf16 matmuls)."""
    sl = slice(r0, r0 + nr)
    base = 0 if r0 < 32 else 32
    wb = (r0 + nr) - base
    bsl = slice(base, base + wb)
    nc.scalar.activation(exp_sb[bsl, :], scores_sb[bsl, :], AF.Exp, accum_out=sums[bsl, :])
    nc.vector.reciprocal(recip[bsl, :], sums[bsl, :])
    nc.vector.tensor_scalar_mul(attn_bf[bsl, :], exp_sb[bsl, :], recip[bsl, :])
    nc.vector.tensor_scalar_mul(attn_sb[bsl, :], exp_sb[bsl, :], recip[bsl, :])
    nc.sync.dma_start(attn_d[sl, :], attn_sb[sl, :])

    for ic in range(IC):
        pt = psum.tile([P, wb], BF16, tag="tp", bufs=2, name=f"pt_at_{r0}_{ic}")
        nc.tensor.transpose(
            pt[:], attn_bf[bsl, ic * P : (ic + 1) * P], ident_bf[bsl, base : base + wb]
        )
        nc.vector.tensor_copy(attnT_bf[:, ic, sl], pt[:, r0 - base : r0 - base + nr])

    # mix^T
    for ac in range(AC):
        pm = psum.tile([P, nr], F32, tag="sm", name=f"pm_{r0}_{ac}")
        for ic in range(IC):
            nc.tensor.matmul(
                pm[:],
                ctx_bf[:, ic, ac * P : (ac + 1) * P],
                attnT_bf[:, ic, sl],
                start=(ic == 0),
                stop=(ic == IC - 1),
            )
        nc.vector.tensor_copy(mixT_bf[:, ac, sl], pm[:])


def _final_project_partial(nc, outT_bf, out_w_bf, psum):
    """accumulate the output^T chunks (host-provided) into the final PSUM
    while the later epilogues run."""
    po = psum.tile([OUT_LEN, DEC], F32, tag="mm", bufs=2, name="po_final")
    for ec in range(EC):
        nc.tensor.matmul(
            po[:], outT_bf[:, ec, :], out_w_bf[:, EC + ec, :],
            start=(ec == 0), stop=False,
        )
    return po


def _final_project_rest(nc, po, mixT_bf, out_w_bf, ones_bf, outb_row_bf,
                        out_sb, out_d):
    for ac in range(AC):
        nc.tensor.matmul(
            po[:], mixT_bf[:, ac, :], out_w_bf[:, ac, :],
            start=False, stop=False,
        )
    nc.tensor.matmul(po[:], ones_bf[:], outb_row_bf[:], start=False, stop=True)
    nc.scalar.activation(out_sb[:], po[:], AF.Tanh)
    nc.sync.dma_start(out_d[:], out_sb[:])


def _build_body(tc):
    nc = tc.nc

    # ---- DRAM I/O (per-core shard shapes; weights host-cast to bf16) ----
    outT_d = nc.dram_tensor("output_t_bf", [DEC, OUT_LEN], BF16, kind="ExternalInput").ap()
    ctx_d = nc.dram_tensor("context_bf", [IN_LEN, ATTN], BF16, kind="ExternalInput").ap()
    ctxT_d = nc.dram_tensor("context_t_bf", [ATTN, IN_LEN], BF16, kind="ExternalInput").ap()
    dec_w_d = nc.dram_tensor("dec_w_bf", [DEC, DEC], BF16, kind="ExternalInput").ap()
    dec_b_d = nc.dram_tensor("dec_b", [DEC, 1], F32, kind="ExternalInput").ap()
    attn_w_d = nc.dram_tensor("attn_w_bf", [ATTN, DEC], BF16, kind="ExternalInput").ap()
    attn_b_d = nc.dram_tensor("attn_b", [ATTN, 1], F32, kind="ExternalInput").ap()
    query_w_d = nc.dram_tensor("query_w", [DEC, 1], F32, kind="ExternalInput").ap()
    out_w_d = nc.dram_tensor("out_w_bf", [ATTN + DEC, DEC], BF16, kind="ExternalInput").ap()
    out_b_d = nc.dram_tensor("out_b", [DEC, 1], F32, kind="ExternalInput").ap()
    out_d = nc.dram_tensor("out", [OUT_LEN, DEC], F32, kind="ExternalOutput").ap()
    attn_d = nc.dram_tensor("attn", [OUT_LEN, IN_LEN], F32, kind="ExternalOutput").ap()

    from contextlib import ExitStack

    with ExitStack() as ctx:
        const = ctx.enter_context(tc.tile_pool(name="const", bufs=1))
        statics = ctx.enter_context(tc.tile_pool(name="statics", bufs=1))
        epool = ctx.enter_context(tc.tile_pool(name="epool", bufs=3))
        fpool = ctx.enter_context(tc.tile_pool(name="fpool", bufs=3))
        spool = ctx.enter_context(tc.tile_pool(name="spool", bufs=2))
        psum = ctx.enter_context(tc.tile_pool(name="psum", bufs=2, space="PSUM"))

        # ---------------- constants / small inputs ----------------
        ident = const.tile([P, P], F32)
        make_identity(nc, ident)
        ident_bf = const.tile([P, P], BF16)
        nc.vector.tensor_copy(ident_bf[:], ident[:])

        # HAM warmup: real matmul activity flips the PE clock gate to 2.4GHz
        wu = psum.tile([P, P], F32, tag="mm", bufs=2)
        for _ in range(16):
            nc.tensor.matmul(wu[:], ident_bf[:], ident_bf[:], start=True, stop=True)

        # ------- critical-path DMAs first (all bf16, host-prepped) -------
        ctxT_bf = statics.tile([P, AC, IN_LEN], BF16)    # [a%, ac, i]
        attn_w_bf = statics.tile([P, AC, DEC], BF16)     # [a%, ac, d]
        dec_w_bf = statics.tile([P, EC, DEC], BF16)      # [e%, ec, d]
        outT_bf = statics.tile([P, EC, OUT_LEN], BF16)   # [e%, ec, o]
        ctx_bf = statics.tile([P, IC, ATTN], BF16)       # [i%, ic, a] (epilogue)
        out_w_bf = statics.tile([P, CC, DEC], BF16)      # [c%, cc, d] (final)
        for ac in range(AC):
            nc.sync.dma_start(ctxT_bf[:, ac, :], ctxT_d[ac * P : (ac + 1) * P, :])
        for ac in range(AC):
            nc.scalar.dma_start(attn_w_bf[:, ac, :], attn_w_d[ac * P : (ac + 1) * P, :])
        for ec in range(EC):
            nc.sync.dma_start(dec_w_bf[:, ec, :], dec_w_d[ec * P : (ec + 1) * P, :])
        for ec in range(EC):
            nc.gpsimd.dma_start(outT_bf[:, ec, :], outT_d[ec * P : (ec + 1) * P, :])
        attn_bias = const.tile([P, DC], F32)
        dec_bias = const.tile([P, DC], F32)
        qw_f = const.tile([P, DC], F32)
        qw_bf = const.tile([P, DC], BF16)
        for tile_, dram_ in ((attn_bias, attn_b_d), (dec_bias, dec_b_d),
                             (qw_f, query_w_d)):
            nc.scalar.dma_start(
                tile_[:], dram_.rearrange("(dc p) one -> p dc one", p=P)
            )
        nc.vector.tensor_copy(qw_bf[:], qw_f[:])

        ones_bf = const.tile([1, OUT_LEN], BF16)
        nc.vector.memset(ones_bf[:], 1.0)
        outb_row_f = const.tile([1, DEC], F32)
        nc.scalar.dma_start(outb_row_f[:], out_b_d.rearrange("d one -> one d"))
        outb_row_bf = const.tile([1, DEC], BF16)
        nc.vector.tensor_copy(outb_row_bf[:], outb_row_f[:])

        # bridge matmuls: keep the PE HAM-busy while DMAs land (paced by deps)
        for ac in range(AC):
            wub = psum.tile([P, IN_LEN], F32, tag="mm", bufs=2, name=f"wub_{ac}")
            nc.tensor.matmul(wub[:], ident_bf[:], ctxT_bf[:, ac, :], start=True, stop=True)

        # ---------------- A^T ----------------
        ATb = statics.tile([P, DC, IN_LEN], BF16)      # [d%, dc, i]
        for dc in range(DC):
            pa = psum.tile([P, IN_LEN], F32, tag="mm", bufs=2, name=f"pa_{dc}")
            for ac in range(AC):
                nc.tensor.matmul(
                    pa[:],
                    attn_w_bf[:, ac, dc * P : (dc + 1) * P],
                    ctxT_bf[:, ac, :],
                    start=(ac == 0),
                    stop=(ac == AC - 1),
                )
            nc.vector.tensor_scalar_add(ATb[:, dc, :], pa[:], attn_bias[:, dc : dc + 1])

        # ---------------- O^T ----------------
        OTb = statics.tile([P, DC, OUT_LEN], F32)      # [d%, dc, o]
        for dc in range(DC):
            po = psum.tile([P, OUT_LEN], F32, tag="sm", name=f"po_{dc}")
            for ec in range(EC):
                nc.tensor.matmul(
                    po[:],
                    dec_w_bf[:, ec, dc * P : (dc + 1) * P],
                    outT_bf[:, ec, :],
                    start=(ec == 0),
                    stop=(ec == EC - 1),
                )
            nc.vector.tensor_scalar_add(OTb[:, dc, :], po[:], dec_bias[:, dc : dc + 1])

        # epilogue-only inputs (1.5MB bf16): gate on OTb so they don't steal
        # DMA bandwidth from the score-path transfers (a bare dma_start has
        # no input semaphores and would fire immediately)
        nc.vector.tensor_copy(ctx_bf[0:1, 0, 0:1], OTb[0:1, 0, 0:1])
        nc.vector.tensor_copy(out_w_bf[0:1, 0, 0:1], OTb[0:1, 0, 0:1])
        nc.sync.dma_start(ctx_bf[:], ctx_d.rearrange("(ic p) a -> p ic a", p=P))
        nc.sync.dma_start(out_w_bf[:], out_w_d.rearrange("(cc p) d -> p cc d", p=P))

        # zero-padded stationary operands: QZ[:, dc, j] is [128, G] with
        # query_w[dc] in column j -> matmul j deposits scores for o_j into
        # PSUM row j, rows != j accumulate zeros.
        QZ = const.tile([P, DC, G, G], BF16)
        nc.gpsimd.memset(QZ[:], 0.0)
        for j in range(G):
            nc.gpsimd.tensor_copy(QZ[:, :, j, j], qw_bf[:, :])

        # ---------------- main loop: adds (DVE) + tanh (ACT) + q-reduce ----
        scores_sb = statics.tile([OUT_LEN, IN_LEN], F32)
        # rows 32..47 are read (as defined garbage) by the rows-48..63
        # quarter epilogue before group 3 writes them
        nc.gpsimd.memset(scores_sb[:], 0.0)
        exp_sb = statics.tile([OUT_LEN, IN_LEN], F32)
        sums = statics.tile([OUT_LEN, 1], F32)
        recip = statics.tile([OUT_LEN, 1], F32)
        attn_sb = statics.tile([OUT_LEN, IN_LEN], F32)
        attn_bf = statics.tile([OUT_LEN, IN_LEN], BF16)
        attnT_bf = statics.tile([P, IC, OUT_LEN], BF16)
        mixT_bf = statics.tile([P, AC, OUT_LEN], BF16)
        out_sb = statics.tile([OUT_LEN, DEC], F32)

        sm_args = (ident_bf, scores_sb, exp_sb, sums, recip, attn_sb, attn_bf,
                   attnT_bf, ctx_bf, mixT_bf, psum, attn_d)

        # group -> o-block mapping: the LAST group handles rows 32..47 so its
        # matmuls can target PSUM partitions 32..47 (tile_position col 32) and
        # a same-partition DVE copy lands the scores without a scatter DMA.
        OBASE = (0, G, 3 * G, 2 * G)
        for og in range(NG):
            last = og == NG - 1
            ps8 = psum.tile([3 * G, IN_LEN], F32, tag="sc", bufs=2, name=f"ps8_{og}")
            if last:
                pview = ps8[2 * G : 3 * G, :]
            else:
                pview = ps8[0:G, :]

            for dc in range(DC):
                E = epool.tile([P, G, IN_LEN], BF16, tag="E", name=f"E_{og}_{dc}")
                for j in range(G):
                    o = OBASE[og] + j
                    nc.vector.tensor_scalar_add(
                        E[:, j, :], ATb[:, dc, :], OTb[:, dc, o : o + 1]
                    )
                Fc = fpool.tile([P, G, IN_LEN], BF16, tag="F", name=f"F_{og}_{dc}")
                if og == 0 and dc == 0:
                    # fine splits: ACT starts after only 4 adds
                    nc.scalar.activation(Fc[:, 0:4], E[:, 0:4], AF.Tanh)
                    nc.scalar.activation(Fc[:, 4:8], E[:, 4:8], AF.Tanh)
                    nc.scalar.activation(Fc[:, 8:16], E[:, 8:16], AF.Tanh)
                elif last and dc == DC - 1:
                    # short tail: only 8 matmuls trail the last tanh piece
                    nc.scalar.activation(Fc[:, 0:8], E[:, 0:8], AF.Tanh)
                    nc.scalar.activation(Fc[:, 8:16], E[:, 8:16], AF.Tanh)
                else:
                    nc.scalar.activation(Fc[:], E[:], AF.Tanh)
                for j in range(G):
                    nc.tensor.matmul(
                        pview,
                        QZ[:, dc, j],
                        Fc[:, j],
                        start=(dc == 0 and j == 0),
                        stop=(dc == DC - 1 and j == G - 1),
                        tile_position=(0, 2 * G) if last else None,
                    )
                if last:
                    # keepalives hold the PE p-state through the tail
                    for k in range(16):
                        wk = psum.tile([P, P], F32, tag="mm", bufs=2,
                                       name=f"wk_{dc}_{k}")
                        nc.tensor.matmul(wk[:], ident_bf[:], ident_bf[:],
                                         start=True, stop=True)
                if last and dc == 1:
                    # rows 48..63 (group 2) complete: quarter epilogue's exp
                    # slots between group-3 tanh units
                    _epilogue_softmax_mix(nc, 3 * G, G, *sm_args)

            if last:
                nc.vector.tensor_copy(scores_sb[2 * G : 3 * G, :], pview)
            elif og == 0:
                nc.vector.tensor_copy(scores_sb[0:G, :], pview)
            else:
                stage8 = spool.tile([G, IN_LEN], F32, tag="st", name=f"stage8_{og}")
                nc.vector.tensor_copy(stage8[:], pview)
                nc.sync.dma_start(
                    scores_sb[OBASE[og] : OBASE[og] + G, :], stage8[:]
                )

            if og == 2:
                # rows 0..31 complete: their softmax + mix runs under groups
                # 2/3 (placed here so the ACT stream never blocks)
                _epilogue_softmax_mix(nc, 0, 2 * G, *sm_args)

        po_final = _final_project_partial(nc, outT_bf, out_w_bf, psum)
        _epilogue_softmax_mix(nc, 2 * G, G, *sm_args)
        _final_project_rest(nc, po_final, mixT_bf, out_w_bf, ones_bf,
                            outb_row_bf, out_sb, out_d)


_CACHE = {}


def build_nc():
    if "nc" in _CACHE:
        return _CACHE["nc"]
    nc = bacc.Bacc(
        "TRN2",
        target_bir_lowering=False,
        debug=False,
        num_devices=N_CORES,
    )
    with tile.TileContext(nc) as tc:
        _build_body(tc)
    nc.compile()
    _CACHE["nc"] = nc
    return nc


def kernel(**inputs):
    nc = build_nc()

    f = lambda k: np.ascontiguousarray(np.asarray(inputs[k], dtype=np.float32))
    bf = lambda a: np.ascontiguousarray(np.asarray(a, dtype=BF16_NP))
    output = f("output")
    context = f("context")
    shared = {
        "dec_w_bf": bf(inputs["dec_w"]),
        "dec_b": f("dec_b").reshape(DEC, 1),
        "attn_w_bf": bf(inputs["attn_w"]),
        "attn_b": f("attn_b").reshape(ATTN, 1),
        "query_w": f("query_w").reshape(DEC, 1),
        "out_w_bf": bf(inputs["out_w"]),
        "out_b": f("out_b").reshape(DEC, 1),
    }
    in_maps = []
    for b in range(N_CORES):
        m = dict(shared)
        m["output_t_bf"] = bf(output[b].T)
        m["context_bf"] = bf(context[b])
        m["context_t_bf"] = bf(context[b].T)
        in_maps.append(m)

    res = bass_utils.run_bass_kernel_spmd(nc, in_maps, core_ids=list(range(N_CORES)))
    _CACHE["last_results"] = res
    out = np.stack([res.results[b]["out"] for b in range(N_CORES)])
    attn = np.stack([res.results[b]["attn"] for b in range(N_CORES)])
    return out, attn
